# revision 31
# baseline (speedup 1.0000x reference)
"""Trainium2 Bass kernel for nn_CRFLoss (single-path CRF numerator loss).

Math (matches the reference):
  loss = ( sum_b [ emis_b + lm_b ] ) / num_tokens
  emis_b = sum over valid positions p of log_probs[b, p, labels[b,p]]
  lm_b   = start_lp[s0] + sum_t trans[s_{t-1}, s_t] + fin[s_{T-1}]
           over the sequence of valid labels (s = label - 1)

Device strategy (pure data parallel over batch, 8 rows per core):
  * positions laid out as pos = p*64 + f  (partition p holds 64 consecutive
    positions per row -> fully contiguous DMA of log_probs)
  * log_probs DMA'd in 2-row chunks with SWDGE f32->bf16 cast directly into
    the matmul rhs tiles (no on-chip convert pass, 3 manually rotated bufs)
  * iota/identity constants generated on device (DVE iota + affine_select,
    ACT broadcast-copy); only the emission selector + packed A-table DMA'd
  * one-hot of labels (42 states, bf16) built with DVE is_equal (2x mode),
    two rows per instruction
  * "previous valid label" via encoded running max:
        enc = (pos*64 + label) * valid    (0 = "nothing yet")
    in-partition prefix scan with tensor_tensor_scan(max), cross-partition
    carry via PE transpose + scan + shifted transpose back;
    prev_label = running_max & 63 (int32 cast + bitwise_and)
  * everything accumulates into one PSUM tile through 512 matmuls,
    col-tiled 2x across the PE array (even j -> partitions 0..41,
    odd j -> partitions 64..105):
        psum[c1, 0, c2] += sum_pos onehot[pos,c1] * lp_bf16[pos,c2]
        psum[c1, 1, c3] += sum_pos onehot[pos,c1] * onehot_prev[pos,c3]
    trace of block 0 = emission sum;  block 1 = transition pair counts
  * A_scores log-softmax computed on device (host only re-packs A into a
    padded [43, 48] table = pure layout); softmax + first/last-label work
    interleaved mid-stream so the tail after the last matmul is short
  * final dot products + first/last-label terms assembled into a [128, 4]
    column tile, reduced with a single ones-matmul -> out[4] per core:
        out = [mainA, mainB+start, fin, num_tokens]
  * host: loss = sum_cores(out0+out1+out2) / sum_cores(out3)
"""

import os
import sys

if "/opt/trn_rl_repo" not in sys.path:
    sys.path.insert(0, "/opt/trn_rl_repo")

COLTILE = int(os.environ.get("COLTILE", "1"))
# ACT pexp reads the int32 masked encoding directly (skip f32 convert)
ACTINT = int(os.environ.get("ACTINT", "1"))

import numpy as np

import concourse.bass as bass
import concourse.tile as tile
from concourse import bacc, mybir
from concourse.bass_utils import run_bass_kernel_spmd

# Problem dims (hardcoded per contract)
B, S, C = 64, 8192, 48
L = 42
IGNORE = -100
N_CORES = 8
B_LOC = B // N_CORES  # 8 rows per core
P = 128               # partitions
F = S // P            # 64 positions per partition per row
RPC = 2               # rows per DMA chunk
NCHUNK = B_LOC // RPC
BIG = float(1 << 23)  # sentinel for min-scan; exact in fp32, BIG % 64 == 0

f32 = mybir.dt.float32
bf16 = mybir.dt.bfloat16
i32 = mybir.dt.int32
Alu = mybir.AluOpType
Act = mybir.ActivationFunctionType
Axis = mybir.AxisListType

# f32 const blob layout
F32_W = 0              # [P, 96]   emission selector (rows 0:42 and 64:106)
F32_TABLE = 96         # [43, 48]  padded A-scores table
F32_TOT = 144

_PROGRAM_CACHE = {}


def _host_constants():
    """Data-independent constant tables shipped to each core (1 small blob)."""
    blob32 = np.zeros((P, F32_TOT), dtype=np.float32)
    # emission diag selector: psum[c1, 0, c2] pairs state c1 with class c2;
    # the gold class for state c1 is c1+1.  Replicated at partitions 64..105
    # for the second col-tile's accumulator.
    for c1 in range(L):
        blob32[c1, F32_W + c1 + 1] = 1.0
        blob32[64 + c1, F32_W + c1 + 1] = 1.0
    return {"blob32": blob32}


def _pack_a_table(A_scores):
    """Pure layout: pad A into the [43, 48] per-state table (no math)."""
    t = np.full((P, 48), -1.0e30, dtype=np.float32)
    t[0, 0:L] = A_scores[0:L]
    t[1:L + 1, 0:L + 1] = A_scores[L:].reshape(L, L + 1)
    t[L + 1:, :] = 0.0
    return t


def build_program():
    """Build the per-core Bass/Tile program (SPMD; every core runs this)."""
    nc = bacc.Bacc("TRN2")

    lp_d = nc.declare_dram_parameter("lp", [B_LOC, S, C], f32, isOutput=False)
    lab_d = nc.declare_dram_parameter("labels", [P, B_LOC, F], i32, isOutput=False)
    b32_d = nc.declare_dram_parameter("blob32", [P, F32_TOT], f32, isOutput=False)
    out_d = nc.declare_dram_parameter("out", [4], f32, isOutput=True)

    with tile.TileContext(nc) as tc:
        with (
            tc.tile_pool(name="const", bufs=1) as cpool,
            tc.tile_pool(name="lab", bufs=1) as lpool,
            tc.tile_pool(name="rhs", bufs=1) as rhspool,
            tc.tile_pool(name="ohn", bufs=3) as ohnpool,
            tc.tile_pool(name="prev", bufs=3) as prevpool,
            tc.tile_pool(name="psum", bufs=1, space=bass.MemorySpace.PSUM) as ppool,
        ):
            # ---------------- inputs in ----------------
            # labels alone on the sync/HWDGE ring (fastest path, tiny);
            # const blob leads the gpsimd/SWDGE ring so it completes before
            # the big lp streams that share it (per-ring FIFO ordering).
            lab = lpool.tile([P, B_LOC, F], i32, tag="lab")
            nc.sync.dma_start(lab[:], lab_d[:])
            blob32 = cpool.tile([P, F32_TOT], f32, tag="blob32")
            nc.gpsimd.dma_start(blob32[:], b32_d[:])
            W = blob32[:, F32_W:F32_W + 96]
            table = blob32[0:43, F32_TABLE:F32_TABLE + 48]

            # ---------------- on-device constants ----------------
            iota48 = cpool.tile([P, 48], bf16, tag="iota48")
            nc.gpsimd.iota(
                iota48[:], [[1, 48]], base=1, channel_multiplier=0,
                allow_small_or_imprecise_dtypes=True,
            )
            # iotap[p, f] = (p*64 + f) * 64
            iotap = cpool.tile([P, F], f32, tag="iotap")
            nc.gpsimd.iota(
                iotap[:], [[64, F]], base=0, channel_multiplier=64 * F,
                allow_small_or_imprecise_dtypes=True,
            )
            ones = cpool.tile([P, 1], f32, tag="ones")
            nc.vector.memset(ones[:], 1.0)
            id128 = cpool.tile([P, P], f32, tag="id128")
            nc.gpsimd.affine_select(
                id128[:],
                ones[:, 0:1].broadcast_to([P, P]),
                [[-1, P]],
                compare_op=Alu.is_equal,
                fill=0.0,
                base=0,
                channel_multiplier=1,
            )
            iotax = cpool.tile([P, L, F], bf16, tag="iotax")

            # ---------------- label prep (DVE) ----------------
            labbf = lpool.tile([P, B_LOC, F], bf16, tag="labbf")
            nc.scalar.copy(labbf[:], lab[:])
            validf = lpool.tile([P, B_LOC, F], f32, tag="validf")
            nc.vector.tensor_scalar(validf[:], lab[:], 0.0, None, op0=Alu.is_gt)
            encb = lpool.tile([P, B_LOC, F], f32, tag="encb")
            iotap_b = iotap[:].unsqueeze(1).broadcast_to([P, B_LOC, F])
            nc.vector.tensor_tensor(encb[:], lab[:], iotap_b, op=Alu.add)
            enc = lpool.tile([P, B_LOC, F], f32, tag="enc")
            nc.vector.tensor_tensor(enc[:], encb[:], validf[:], op=Alu.mult)

            # ---------------- scans ----------------
            # scano[:, r, 0] = 0; scano[:, r, 1+k] = max(enc[:, r, 0..k])
            scano = lpool.tile([P, B_LOC, F + 1], f32, tag="scano")
            nc.vector.memset(scano[:, :, 0:1], 0.0)
            for r in range(B_LOC):
                nc.vector.tensor_tensor_scan(
                    scano[:, r, 1 : F + 1],
                    enc[:, r, :],
                    enc[:, r, :],
                    0.0,
                    op0=Alu.max,
                    op1=Alu.max,
                )
            stats = lpool.tile([P, 96], f32, tag="stats")
            nc.vector.tensor_copy(stats[:, 0:B_LOC], scano[:, :, F])
            # critical-path transpose: per-partition running maxima only
            pstatsA = ppool.tile([8, P], f32, tag="pstatsA")
            nc.tensor.transpose(pstatsA[:], stats[:, 0:8], id128[:])
            # EXCLUSIVE running max of per-partition maxima, per row:
            # scanT[r, p] = max over partitions < p
            scanT = lpool.tile([8, P], f32, tag="scanT")
            nc.vector.memset(scanT[:, 0:1], 0.0)
            nc.vector.tensor_tensor_scan(
                scanT[:, 1:P],
                pstatsA[0:8, 0 : P - 1],
                id128[0:8, 0 : P - 1],
                0.0,
                op0=Alu.max,
                op1=Alu.bypass,
            )
            # back into [128, 8] per-partition carry
            pP = ppool.tile([P, 8], f32, tag="pP")
            nc.tensor.transpose(pP[:], scanT[:], id128[0:8, 0:8])
            # prev_enc = max(in-partition exclusive scan, cross-part carry),
            # then prev label = enc & 63 -- all 8 rows in 3 batched ops
            prevb = lpool.tile([P, B_LOC, F], f32, tag="prevb")
            nc.vector.tensor_tensor(
                prevb[:],
                scano[:, :, 0:F],
                pP[:].unsqueeze(2).broadcast_to([P, B_LOC, F]),
                op=Alu.max,
            )
            previ = lpool.tile([P, B_LOC, F], i32, tag="previ")
            nc.vector.tensor_copy(previ[:], prevb[:])
            prevm = lpool.tile([P, B_LOC, F], i32, tag="prevm")
            nc.vector.tensor_scalar(prevm[:], previ[:], 63, None, op0=Alu.bitwise_and)

            # A-scores log-softmax pieces (emitted interleaved with the main
            # loop below so neither DVE nor ACT stalls at the head of a FIFO)
            tmax = lpool.tile([43, 1], f32, tag="tmax")
            x1 = lpool.tile([43, 48], f32, tag="x1")
            ex = lpool.tile([43, 48], f32, tag="ex")
            ssum = lpool.tile([43, 1], f32, tag="ssum")
            lsum = lpool.tile([43, 1], f32, tag="lsum")
            lse = lpool.tile([43, 1], f32, tag="lse")
            tls = lpool.tile([43, 48], f32, tag="tls")
            ptT = ppool.tile([43, 43], f32, tag="ptT")

            def softmax_part1():
                nc.vector.tensor_reduce(
                    tmax[:], table, axis=Axis.X, op=Alu.max, negate=True
                )
                nc.scalar.activation(x1[:], table, Act.Identity, bias=tmax[:])
                nc.scalar.activation(ex[:], x1[:], Act.Exp)

            def softmax_part2():
                nc.vector.tensor_reduce(ssum[:], ex[:], axis=Axis.X, op=Alu.add)
                nc.scalar.activation(lsum[:], ssum[:], Act.Ln)
                # tmax already holds -max, so -lse = tmax - lsum
                nc.vector.tensor_tensor(lse[:], tmax[:], lsum[:], op=Alu.subtract)
                nc.scalar.activation(tls[:], table, Act.Identity, bias=lse[:])
                # ptT[j, i] = tls[i, j]
                nc.tensor.transpose(ptT[:], tls[0:43, 0:43], id128[0:43, 0:43])
                # W[c1, 48+c3] = trans[state c3 -> state c1] = tls[c3+1, c1]
                nc.vector.tensor_copy(W[0:L, 48:48 + L], ptT[0:L, 1:43])
                if COLTILE:
                    # second col-tile's accumulator needs W at partitions 64+
                    nc.scalar.dma_start(W[64:64 + L, 48:48 + L], W[0:L, 48:48 + L])

            # deferred stats + first/last-label decode, hoisted mid-stream
            encpb = lpool.tile([P, B_LOC, F], f32, tag="encpb")
            encmin = lpool.tile([P, B_LOC, F], f32, tag="encmin")
            pstats = ppool.tile([96, P], f32, tag="pstats")
            Z = lpool.tile([P, 4], f32, tag="Z")
            colv = lpool.tile([P, 1], f32, tag="colv")
            ci = lpool.tile([P, 1], i32, tag="ci")
            cm = lpool.tile([P, 1], i32, tag="cm")
            ohfl = lpool.tile([P, 48], bf16, tag="ohfl")
            sel2 = lpool.tile([P, 2], bf16, tag="sel2")
            histPS = ppool.tile([48, 2], f32, tag="histPS")
            startcol = lpool.tile([43, 1], f32, tag="startcol")
            finsh = lpool.tile([42, 1], f32, tag="finsh")

            def stats_partA():
                nc.scalar.activation(encpb[:], enc[:], Act.Copy, bias=BIG)
                nc.vector.scalar_tensor_tensor(
                    encmin[:], validf[:], -BIG, encpb[:], op0=Alu.mult, op1=Alu.add
                )
                nc.vector.tensor_reduce(
                    stats[:, 32:40], encmin[:], axis=Axis.X, op=Alu.min
                )
                nc.vector.tensor_reduce(
                    stats[:, 64:72], validf[:], axis=Axis.X, op=Alu.add
                )
                nc.vector.memset(Z[:], 0.0)
                nc.vector.memset(colv[:], 0.0)
                nc.vector.memset(sel2[:], 0.0)
                nc.vector.memset(sel2[0:8, 0:1], 1.0)
                nc.vector.memset(sel2[32:40, 1:2], 1.0)

            def stats_partB():
                nc.tensor.transpose(pstats[:], stats[:, 0:96], id128[:])
                # inclusive full-row max = max(excl scan end, last partition)
                nc.vector.tensor_tensor(
                    colv[0:8, 0:1],
                    scanT[:, P - 1 : P],
                    pstatsA[0:8, P - 1 : P],
                    op=Alu.max,
                )
                nc.vector.tensor_reduce(
                    colv[32:40, 0:1], pstats[32:40, :], axis=Axis.X, op=Alu.min
                )
                # first/last labels = (enc encodings) & 63, partition-parallel:
                # rows 0..7 hold last-enc, rows 32..39 first-enc
                nc.vector.tensor_copy(ci[:], colv[:])
                nc.vector.tensor_scalar(cm[:], ci[:], 63, None, op0=Alu.bitwise_and)
                nc.vector.tensor_tensor(
                    ohfl[:],
                    cm[:].broadcast_to([P, 48]),
                    iota48[:],
                    op=Alu.is_equal,
                )
                # histogram matmul: col 0 = last-label counts (rows 0..7),
                # col 1 = first-label counts (rows 32..39); hist index = state
                nc.tensor.matmul(
                    histPS[:], ohfl[:], sel2[:], start=True, stop=True
                )
                # start_lp per state, partition-major (ptT column 0)
                nc.vector.tensor_copy(startcol[:], ptT[0:43, 0:1])
                # fin per state: tls[s+1, 42] shifted down one partition
                nc.scalar.dma_start(finsh[:], tls[1:43, 42:43])
                nc.vector.tensor_tensor(
                    Z[0:43, 1:2], histPS[0:43, 1:2], startcol[:], op=Alu.mult
                )
                nc.vector.tensor_tensor(
                    Z[0:42, 2:3], histPS[0:42, 0:1], finsh[:], op=Alu.mult
                )
                nc.vector.tensor_reduce(
                    Z[64:72, 3:4], pstats[64:72, :], axis=Axis.X, op=Alu.add
                )

            # ---------------- main streaming loop ----------------
            # variable chunk sizes: small first chunks shorten the ramp,
            # small last chunks shrink the final transfer's compute shadow
            CHUNKS = [2, 2, 2, 1, 1]
            assert sum(CHUNKS) == B_LOC
            # one rhs buf per chunk (no reuse -> no write-after-read
            # hazards with the upfront DMA issues); pad cols zeroed once
            rhs_bufs = [
                rhspool.tile([P, rpc, 2, F, C], bf16, name=f"rhs{i}", tag=f"rhs{i}")
                for i, rpc in enumerate([2, 2, 2, 1, 1])
            ]
            # accumulator: even j -> rows 0..41, odd j -> rows 64..105
            pacc = ppool.tile([106 if COLTILE else L, 2, 48], f32, tag="pacc")
            # issue all lp DMAs upfront on the gpsimd ring so the transfers
            # run back-to-back; pad memsets slot in after the first issue
            starts = [sum(CHUNKS[:k]) for k in range(len(CHUNKS))]
            for k, rpc in enumerate(CHUNKS):
                r0 = starts[k]
                nc.gpsimd.dma_start(
                    rhs_bufs[k][:, 0:rpc, 0],
                    lp_d[r0 : r0 + rpc].rearrange("r (p f) c -> p r f c", p=P),
                )
                if k == 0:
                    # iota_exp[p, c, f] = c+1 (class-major, contiguous f)
                    nc.gpsimd.iota(
                        iotax[:], [[1, L], [0, F]], base=1, channel_multiplier=0,
                        allow_small_or_imprecise_dtypes=True,
                    )
            for k, rpc in enumerate(CHUNKS):
                rhs_t = rhs_bufs[k]
                r0 = starts[k]
                # ACT expands prev labels so the is_equal runs 2x packed
                pexp = prevpool.tile([P, RPC, F, C], bf16, tag="pexp")
                nc.scalar.copy(
                    pexp[:, 0:rpc],
                    prevm[:, r0:r0 + rpc].unsqueeze(3).broadcast_to([P, rpc, F, C]),
                )
                nc.vector.tensor_tensor(
                    rhs_t[:, 0:rpc, 1],
                    pexp[:, 0:rpc],
                    iota48[:].unsqueeze(1).unsqueeze(1).broadcast_to(
                        [P, rpc, F, C]
                    ),
                    op=Alu.is_equal,
                )
                # class-major one-hot vs expanded iota const (2x mode)
                ohn = ohnpool.tile([P, RPC, L, F], bf16, tag="ohn")
                nc.vector.tensor_tensor(
                    ohn[:, 0:rpc],
                    labbf[:, r0:r0 + rpc, :].unsqueeze(2).broadcast_to(
                        [P, rpc, L, F]
                    ),
                    iotax[:].unsqueeze(1).broadcast_to([P, rpc, L, F]),
                    op=Alu.is_equal,
                )
                for rr in range(rpc):
                    r = r0 + rr
                    for j in range(F):
                        if COLTILE:
                            tgt = pacc[0:L] if j % 2 == 0 else pacc[64:64 + L]
                            first = r == 0 and j < 2
                            last = r == B_LOC - 1 and j >= F - 2
                        else:
                            tgt = pacc[0:L]
                            first = r == 0 and j == 0
                            last = r == B_LOC - 1 and j == F - 1
                        nc.tensor.matmul(
                            tgt,
                            ohn[:, rr, :, j],
                            rhs_t[:, rr, :, j, :],
                            start=first,
                            stop=last,
                            skip_group_check=True,
                        )
                if k == 0:
                    softmax_part1()
                    softmax_part2()
                elif k == 1:
                    stats_partA()
                elif k == 3:
                    stats_partB()

            # ---------------- tail ----------------
            NP = 106 if COLTILE else L
            psb = lpool.tile([NP, 96], f32, tag="psb")
            if COLTILE:
                # zero the unused middle partitions (32-aligned start; rows
                # 32:42 get overwritten by the copy below)
                nc.vector.memset(psb[32:64, :], 0.0)
            scratch = lpool.tile([NP, 96], f32, tag="scratch")
            pacc_f = pacc[:].rearrange("a b c -> a (b c)")
            nc.vector.tensor_copy(psb[0:L], pacc_f[0:L])
            if COLTILE:
                nc.vector.tensor_copy(psb[64:64 + L], pacc_f[64:64 + L])
            nc.vector.tensor_tensor(
                scratch[0:NP], psb[0:NP], W[0:NP], op=Alu.mult
            )
            nc.vector.tensor_reduce(
                Z[0:NP, 0:1], scratch[0:NP], axis=Axis.X, op=Alu.add
            )
            pout = ppool.tile([4, 1], f32, tag="pout")
            nc.tensor.matmul(pout[:], Z[:], ones[:], start=True, stop=True)
            outsb = lpool.tile([4, 1], f32, tag="outsb")
            nc.vector.tensor_copy(outsb[:], pout[:])
            nc.sync.dma_start(out_d[:], outsb[:])

    nc.finalize()
    return nc


def _get_program():
    if "nc" not in _PROGRAM_CACHE:
        _PROGRAM_CACHE["nc"] = build_program()
    return _PROGRAM_CACHE["nc"]


def make_in_maps(log_probs, A_scores, labels, input_lens):
    consts = _host_constants()
    atab = _pack_a_table(np.asarray(A_scores, dtype=np.float32))
    blob32 = consts["blob32"].copy()
    blob32[:, F32_TABLE:F32_TABLE + 48] = atab
    in_maps = []
    for c in range(N_CORES):
        sl = slice(c * B_LOC, (c + 1) * B_LOC)
        # pre-permute labels to the on-chip layout [p, r, f], pos = p*64+f,
        # so the device DMA is one contiguous chunk per partition
        lab = np.ascontiguousarray(
            np.asarray(labels[sl], dtype=np.int32)
            .reshape(B_LOC, P, F)
            .transpose(1, 0, 2)
        )
        in_maps.append(
            {
                "lp": np.ascontiguousarray(log_probs[sl], dtype=np.float32),
                "labels": lab,
                "blob32": blob32,
            }
        )
    return in_maps


def combine_outputs(outs):
    num = 0.0
    tok = 0.0
    for o in outs:
        o = np.asarray(o, dtype=np.float64)
        num += o[0] + o[1] + o[2]
        tok += o[3]
    return np.float32(num / tok)


def kernel(log_probs, A_scores, labels, input_lens):
    nc = _get_program()
    in_maps = make_in_maps(log_probs, A_scores, labels, input_lens)
    res = run_bass_kernel_spmd(nc, in_maps, list(range(N_CORES)))
    return combine_outputs([res.results[c]["out"] for c in range(N_CORES)])
